# revision 33
# baseline (speedup 1.0000x reference)
"""Causal attention with key padding for Trainium2, sharded over 8 NeuronCores.

Contract: kernel(**inputs) takes the FULL inputs (q, k, v, att_mask, pad_mask)
as numpy arrays and returns the FULL [B, H, L, D] output.

Strategy (v5):
  - Shard the 64 (batch, head) units across 8 cores; each core runs 8 heads.
  - Per head, key chunks are processed in adjacent PAIRS (2i, 2i+1):
    QK^T as two bf16 matmuls per (pair, query-block), one per 64-partition
    row-group half (contract D=64); the HW runs the pair concurrently
    (~213ns per pair of [128,512] score chunks).  Crossing (diagonal)
    chunks are PADDED to the full 512-query width -- dead columns are
    computed but never read, keeping every matmul pair-packable.
  - Causal boundary: a tiny extra matmul ((-1e9*I)^T @ strict_lower_ones)
    ACCUMULATES -1e9 onto the masked triangle of each crossing chunk's
    diagonal block in PSUM, so exp() kills it naturally in every path and
    no post-exp triangle multiply exists.
  - exp() splits across ScalarE (exact exp) and VectorE (Schraudolph via
    one tensor_scalar), cost-balanced at emission time:
      qb<3: bf16 probabilities (qb0 always ScalarE: short rows need exact
        exp); Schraudolph writes int16-bitcast-bf16.
      qb3: fp8e4 probabilities as p' = exp(s*scale - 3.5) (range fits
        fp8e4 for |s*scale| < 8.9); Schraudolph writes uint8-bitcast-fp8e4
        (HW rounds to nearest; saturates negatives to 0 = fp8 +0).
  - PV: for qb3 the pair's P~ and [V|1] are fp8e4 and ONE DoubleRow matmul
    contracts BOTH chunks (256 keys) in ~213ns -- 2x over bf16.  Dead/
    missing-member columns of the dedicated per-(pair,qb) fp8 slots are
    zeroed once at startup and persist across heads.  qb0..2 use bf16 PV
    per chunk (fp8 V noise fails the tolerance on concentrated rows
    there).  Padded keys have zeroed V rows and ones column, so padding
    costs nothing on device.
  - PSUM: 2 accumulator banks + 3 double-width score slots; qb2/qb3 PV
    starts are delayed (slot-granular software pipelining with per-qb
    FIFO release queues) so 2 accumulator banks always suffice.
  - [65, 512] accumulators (nums + denominator row) copy to SBUF (sc/ve
    balanced) and DMA out per head.  Normalization (num/den) and the
    final [D, L] -> [L, D] transpose happen on the host.
"""

import numpy as np

N_CORES = 8
KC = 128          # key-chunk (partition) size
QB = 512          # query-block width

_LOG2E = 1.4426950408889634
_EXP_C = 0.04305   # centers the log-linear interpolation error
_FP8_BIAS = 3.5    # p' = exp(s*scale - 3.5): keeps p' in fp8e4 range


# --------------------------------------------------------------------------
# numpy fallback (exact reference math) -- only used if the input masks do
# not match the causal + suffix-pad structure this kernel specializes to.
# --------------------------------------------------------------------------
def _reference_np(q, k, v, att_mask, pad_mask):
    B, H, L, D = q.shape
    scale = np.float32(1.0) / np.sqrt(np.float32(D))
    out = np.empty_like(q)
    for b in range(B):
        for h in range(H):
            att = (q[b, h] @ k[b, h].T) * scale
            att = att + att_mask[0, 0]
            att = np.where(pad_mask[b][None, :], -np.inf, att)
            att = att - att.max(axis=-1, keepdims=True)
            p = np.exp(att)
            p = p / p.sum(axis=-1, keepdims=True)
            out[b, h] = p @ v[b, h]
    return out


# --------------------------------------------------------------------------
# Bass program builder
# --------------------------------------------------------------------------
def _build_program(NH, L, D, skip):
    """Build the per-core SPMD Bass program.

    NH: heads per core.  L: sequence length.  D: head dim (must be 64).
    skip: frozenset of fully-padded key chunks (never computed).
    """
    import os

    import concourse.bacc as bacc
    import concourse.mybir as mybir
    import concourse.tile as tile

    f32 = mybir.dt.float32
    bf16 = mybir.dt.bfloat16
    fp8 = mybir.dt.float8e4
    u8 = mybir.dt.uint8
    i16 = mybir.dt.int16
    DR = mybir.MatmulPerfMode.DoubleRow

    NCH = L // KC          # 16 key chunks
    NQB = L // QB          # 4 query blocks
    CPB = QB // KC         # 4 chunks per query block
    NPAIR = (NCH + 1) // 2
    VO_W = 80              # fp8 weight stride (%16==0), cols 65..79 zero
    scale = float(1.0 / np.sqrt(np.float32(D)))
    exp_a16 = float(128.0 * _LOG2E * scale)
    exp_b16 = float(128.0 * (127.0 - _EXP_C))
    exp_a8 = float(8.0 * _LOG2E * scale)
    exp_b8 = float(8.0 * (7.0 - _EXP_C) - 8.0 * _FP8_BIAS * _LOG2E)

    ve_bias = float(os.environ.get("KVE_BIAS", "0"))
    n_warm = int(os.environ.get("KWARM", "28"))
    lag = int(os.environ.get("KLAG", "4"))
    qbmin8 = int(os.environ.get("KFP8_QBMIN", "3"))  # fp8 PV for qb >= this

    def fq(c):
        return c // CPB    # first query block needing chunk c

    def members_of(p):
        return [
            (j, 2 * p + j) for j in (0, 1)
            if 2 * p + j < NCH and (2 * p + j) not in skip
        ]

    def last_chunk(qb):
        cmax = min(CPB * qb + CPB - 1, NCH - 1)
        while cmax in skip:
            cmax -= 1
        return cmax

    def last_pair(qb):
        cmax = CPB * qb + CPB - 1
        for pp in range(min(cmax // 2, NPAIR - 1), -1, -1):
            if members_of(pp):
                return pp
        return 0

    nc = bacc.Bacc("TRN2", target_bir_lowering=False, debug=False)

    qt_d = nc.dram_tensor("qt", [NH, D, L], bf16, kind="ExternalInput")
    kt_d = nc.dram_tensor("kt", [NH, D, L], bf16, kind="ExternalInput")
    vo8_d = nc.dram_tensor("vo8", [NH, KC, NPAIR, 2, VO_W], fp8,
                           kind="ExternalInput")
    vo0_d = nc.dram_tensor("vo0", [NH, KC, NCH, D + 1], bf16,
                           kind="ExternalInput")
    negi_d = nc.dram_tensor("negi", [KC, KC], bf16, kind="ExternalInput")
    utri_d = nc.dram_tensor("utri", [KC, 2 * KC], bf16, kind="ExternalInput")
    out_d = nc.dram_tensor("out", [NH, D + 1, L], bf16, kind="ExternalOutput")

    load = {"sc": 0.0, "ve": 0.0}

    def route(cost_sc, cost_ve):
        if load["sc"] + cost_sc <= load["ve"] + cost_ve + ve_bias:
            load["sc"] += cost_sc
            return "sc"
        load["ve"] += cost_ve
        return "ve"

    with tile.TileContext(nc) as tc:
        with (
            tc.tile_pool(name="consts", bufs=1) as consts,
            tc.tile_pool(name="ktp", bufs=2) as ktp,
            tc.tile_pool(name="qtp", bufs=2) as qtp,
            tc.tile_pool(name="vo8p", bufs=2) as vo8p,
            tc.tile_pool(name="vo0p", bufs=2) as vo0p,
            tc.tile_pool(name="pt8c", bufs=1) as pt8c,
            tc.tile_pool(name="pt0p", bufs=12) as pt0p,
            tc.tile_pool(name="osb", bufs=2) as osb,
            tc.tile_pool(name="stg", bufs=3, space="PSUM") as stgp,
            tc.tile_pool(name="acc", bufs=2, space="PSUM") as accp,
        ):
            negi = consts.tile([KC, KC], bf16, tag="negi")
            utri = consts.tile([KC, 2 * KC], bf16, tag="utri")
            bias_m2 = consts.tile([KC, 1], f32, tag="bias")
            nc.vector.memset(bias_m2[:], -_FP8_BIAS)

            # Warm-up with no DMA dependency: sustained PE activity makes
            # the clock gate grant full rate sooner; a dummy exp pulls the
            # ACT table load off the critical path -- all while the first
            # head's tensors stream in.
            wsrc = consts.tile([KC, QB], bf16, tag="wsrc")
            wout = consts.tile([KC, 1], f32, tag="wout")
            nc.vector.memset(wsrc[:], 0.0)
            warm = stgp.tile([KC, 2 * QB], f32, tag="stg")
            for i in range(n_warm):
                nc.tensor.matmul(
                    out=warm[:, 0:QB], lhsT=wsrc[:, 0:KC], rhs=wsrc[:],
                    start=True, stop=True,
                )
                if i == 0:
                    nc.scalar.activation(
                        out=wout[:], in_=warm[:, 0:1],
                        func=mybir.ActivationFunctionType.Exp,
                    )

            nc.sync.dma_start(out=negi[:], in_=negi_d[:])
            nc.sync.dma_start(out=utri[:], in_=utri_d[:])

            # Dedicated fp8 probability slots per (pair, qb>=1).  Dead
            # columns (crossing-chunk heads, missing pair members) are
            # zeroed ONCE here; exp rewrites exactly the live columns each
            # head, so the zeros persist.
            pt8_slot = {}
            for p in range(NPAIR):
                mem = members_of(p)
                if not mem:
                    continue
                for qb in range(max(qbmin8, fq(2 * p)), NQB):
                    t = pt8c.tile([KC, 2, QB], fp8, tag=f"pt8_{p}_{qb}")
                    pt8_slot[(p, qb)] = t
                    live_j = {j for j, _ in mem}
                    for j in (0, 1):
                        if j not in live_j:
                            nc.gpsimd.memset(t[:, j, :], 0.0)
                            continue
                        c = 2 * p + j
                        dead = max(0, c * KC - qb * QB)
                        if dead > 0:
                            nc.gpsimd.memset(t[:, j, 0:dead], 0.0)

            # ------------------------------------------------------------
            # software-pipelined emission
            # ------------------------------------------------------------
            pv_queue = []    # (release_tick, qb, kind, payload)
            epi_queue = []   # (release_tick, h, qb, acc_of, o_t)
            tick = [0]
            # qb2/qb3 PV starts are delayed so only 2 accumulator banks are
            # ever live (their dedicated pt tiles hold the probabilities);
            # ticks count SLOTS (one (pair, qb) item each)
            pv_extra = {0: 0, 1: 0, 2: 6, 3: 12}

            def emit_pv(job):
                _, qb, kind, payload = job
                if kind == "fp8":
                    p, acc_of, vo8_t, start, stop = payload
                    if qb not in acc_of:
                        acc_of[qb] = accp.tile([D + 1, QB], f32,
                                               name="acc", tag="acc")
                    nc.tensor.matmul(
                        out=acc_of[qb][:],
                        lhsT=vo8_t[:, p, :, 0:D + 1],
                        rhs=pt8_slot[(p, qb)][:, :, :],
                        start=start, stop=stop,
                        perf_mode=DR,
                    )
                else:
                    c, acc_of, vo0_t, pt0, j, dead, start, stop = payload
                    if qb not in acc_of:
                        acc_of[qb] = accp.tile([D + 1, QB], f32,
                                               name="acc", tag="acc")
                    nc.tensor.matmul(
                        out=acc_of[qb][:, dead:QB],
                        lhsT=vo0_t[:, c, :],
                        rhs=pt0[:, j * QB + dead:(j + 1) * QB],
                        start=start, stop=stop,
                    )

            def emit_epi(job):
                _, h, qb, acc_of, o_t = job
                c_sc = (394 + QB) / 1.2
                c_ve = (196 + QB) / 0.96
                dst = o_t[:, qb * QB:(qb + 1) * QB]
                if route(c_sc, c_ve) == "sc":
                    nc.scalar.copy(out=dst, in_=acc_of[qb][:])
                else:
                    nc.vector.tensor_copy(out=dst, in_=acc_of[qb][:])
                if qb == NQB - 1:
                    # last block of head h: ship the whole [65, L] tile out
                    nc.gpsimd.dma_start(out=out_d[h], in_=o_t[:])

            def flush(drain=False):
                # epilogues first: the acc bank is recycled by a later PV
                # allocation, whose emission must come after the copy
                def pop_epis():
                    while epi_queue and (drain or epi_queue[0][0] <= tick[0]):
                        emit_epi(epi_queue.pop(0))

                def pop_pvs():
                    # per-qb FIFO: a qb's job may only run once all earlier
                    # jobs of the same qb have been emitted; emit eligible
                    # jobs round-robin across qbs so consecutive PV matmuls
                    # hit different accumulator banks
                    blocked = set()
                    eligible = []
                    i = 0
                    while i < len(pv_queue):
                        job = pv_queue[i]
                        if job[1] not in blocked and (
                            drain or job[0] <= tick[0] - lag
                        ):
                            eligible.append(pv_queue.pop(i))
                        else:
                            blocked.add(job[1])
                            i += 1
                    byqb = {}
                    for job in eligible:
                        byqb.setdefault(job[1], []).append(job)
                    order = sorted(byqb)
                    while byqb:
                        for qb_ in list(order):
                            if qb_ in byqb:
                                emit_pv(byqb[qb_].pop(0))
                                if not byqb[qb_]:
                                    del byqb[qb_]

                if drain:
                    pop_pvs()
                    pop_epis()
                else:
                    pop_epis()
                    pop_pvs()

            for h in range(NH):
                kt_t = ktp.tile([KC, L], bf16)
                qt_t = qtp.tile([KC, L], bf16)
                vo8_t = vo8p.tile([KC, NPAIR, 2, VO_W], fp8)
                vo0_t = vo0p.tile([KC, NCH, D + 1], bf16)
                col_splits = ((0, QB), (QB, L)) if h == 0 else ((0, L),)
                for lo, hi in col_splits:
                    for half in (0, 1):
                        nc.sync.dma_start(
                            out=kt_t[half * D:(half + 1) * D, lo:hi],
                            in_=kt_d[h, :, lo:hi],
                        )
                        nc.sync.dma_start(
                            out=qt_t[half * D:(half + 1) * D, lo:hi],
                            in_=qt_d[h, :, lo:hi],
                        )
                nc.sync.dma_start(out=vo8_t[:], in_=vo8_d[h])
                nc.sync.dma_start(out=vo0_t[:], in_=vo0_d[h])
                o_t = osb.tile([D + 1, L], bf16)

                acc_of = {}
                qb_started = set()
                for p in range(NPAIR):
                    mem = members_of(p)
                    if not mem:
                        continue
                    f = fq(2 * p)
                    for qb in range(f, NQB):
                        stg = stgp.tile([KC, 2 * QB], f32, tag="stg")
                        for j, c in mem:
                            crossing = c * KC - qb * QB >= 0
                            nc.tensor.matmul(
                                out=stg[:, j * QB:(j + 1) * QB],
                                lhsT=kt_t[j * D:(j + 1) * D,
                                          c * KC:(c + 1) * KC],
                                rhs=qt_t[j * D:(j + 1) * D,
                                         qb * QB:(qb + 1) * QB],
                                start=True, stop=not crossing,
                            )
                        # causal boundary: accumulate -1e9 onto the
                        # strictly-upper triangle of each diagonal block
                        for j, c in mem:
                            dead = c * KC - qb * QB
                            if dead >= 0:
                                nc.tensor.matmul(
                                    out=stg[:, j * QB + dead:
                                            j * QB + dead + KC],
                                    lhsT=negi[:],
                                    rhs=utri[:, 0:KC],
                                    start=False, stop=True,
                                )
                        # exp: contiguous runs in the flat [0, 2*QB) space
                        runs = []
                        for j, c in mem:
                            dead = max(0, c * KC - qb * QB)
                            r0, r1 = j * QB + dead, (j + 1) * QB
                            if runs and runs[-1][1] == r0:
                                runs[-1][1] = r1
                            else:
                                runs.append([r0, r1])
                        if qb < qbmin8:
                            # bf16 path: one [KC, 2*QB] pt tile per pair
                            pt0 = pt0p.tile([KC, 2 * QB], bf16)
                            for r0, r1 in runs:
                                w = r1 - r0
                                c_sc = (394 + w) / 1.2
                                c_ve = (196 + w) / 0.96
                                if qb == 0:
                                    load["sc"] += c_sc
                                    eng = "sc"
                                else:
                                    eng = route(c_sc, c_ve)
                                if eng == "sc":
                                    nc.scalar.activation(
                                        out=pt0[:, r0:r1], in_=stg[:, r0:r1],
                                        func=mybir.ActivationFunctionType.Exp,
                                        scale=scale,
                                    )
                                else:
                                    nc.vector.tensor_scalar(
                                        out=pt0[:, r0:r1].bitcast(i16),
                                        in0=stg[:, r0:r1],
                                        scalar1=exp_a16, scalar2=exp_b16,
                                        op0=mybir.AluOpType.mult,
                                        op1=mybir.AluOpType.add,
                                    )
                            for j, c in mem:
                                dead = max(0, c * KC - qb * QB)
                                extra = (0 if qb in qb_started
                                         else pv_extra.get(qb, 0))
                                qb_started.add(qb)
                                pv_queue.append((
                                    tick[0] + extra, qb, "bf16",
                                    (c, acc_of, vo0_t, pt0, j, dead,
                                     c == 0, c == last_chunk(qb)),
                                ))
                        else:
                            pt8 = pt8_slot[(p, qb)]
                            pt8f = pt8.rearrange("p two q -> p (two q)")
                            for r0, r1 in runs:
                                w = r1 - r0
                                c_sc = (394 + w) / 1.2
                                c_ve = (196 + w) / 0.96
                                if route(c_sc, c_ve) == "sc":
                                    nc.scalar.activation(
                                        out=pt8f[:, r0:r1],
                                        in_=stg[:, r0:r1],
                                        func=mybir.ActivationFunctionType.Exp,
                                        scale=scale, bias=bias_m2[:],
                                    )
                                else:
                                    nc.vector.tensor_scalar(
                                        out=pt8f[:, r0:r1].bitcast(u8),
                                        in0=stg[:, r0:r1],
                                        scalar1=exp_a8, scalar2=exp_b8,
                                        op0=mybir.AluOpType.mult,
                                        op1=mybir.AluOpType.add,
                                    )
                            extra = (0 if qb in qb_started
                                     else pv_extra.get(qb, 0))
                            qb_started.add(qb)
                            pv_queue.append((
                                tick[0] + extra, qb, "fp8",
                                (p, acc_of, vo8_t,
                                 p == 0, p == last_pair(qb)),
                            ))
                        if p == last_pair(qb):
                            epi_queue.append((
                                tick[0] + lag + 1,
                                h, qb, acc_of, o_t,
                            ))
                        tick[0] += 1
                        flush()

            flush(drain=True)
            flush(drain=True)
    if os.environ.get("KDEBUG_ROUTE"):
        print(f"route loads: sc={load['sc']:.0f}ns ve={load['ve']:.0f}ns "
              f"(ve_bias={ve_bias:.0f})")
    nc.finalize()
    return nc


# --------------------------------------------------------------------------
# host-side wrapper
# --------------------------------------------------------------------------
_PROG_CACHE = {}


def _get_program(NH, L, D, skip):
    key = (NH, L, D, skip)
    if key not in _PROG_CACHE:
        _PROG_CACHE[key] = _build_program(NH, L, D, skip)
    return _PROG_CACHE[key]


def _causal_ok(att_mask, L):
    if att_mask.shape != (1, 1, L, L):
        return False
    m = att_mask[0, 0]
    iu = np.triu_indices(L, 1)
    if not np.all(m[iu] == np.float32(-1e9)):
        return False
    il = np.tril_indices(L)
    return bool(np.all(m[il] == 0.0))


def kernel(q, k, v, att_mask, pad_mask):
    import ml_dtypes

    from concourse.bass_utils import run_bass_kernel_spmd

    B, H, L, D = q.shape
    U = B * H
    NCH = L // KC
    CPB = QB // KC
    NPAIR = (NCH + 1) // 2
    VO_W = 80
    if (
        U % N_CORES != 0
        or L % QB != 0
        or D != 64
        or not _causal_ok(att_mask, L)
    ):
        return _reference_np(q, k, v, att_mask, pad_mask)

    NH = U // N_CORES  # units (heads) per core

    pad = np.asarray(pad_mask, dtype=bool)          # [B, L]
    pad_u = np.repeat(pad, H, axis=0)               # [U, L]

    skip = frozenset(
        kc for kc in range(NCH)
        if np.all(pad_u[:, kc * KC:(kc + 1) * KC])
    )
    per_u_skip = [
        frozenset(
            kc for kc in range(NCH)
            if np.all(pad_u[u, kc * KC:(kc + 1) * KC])
        )
        for u in range(U)
    ]
    # chunks 0..CPB-1 must exist (qb0 bf16 path assumes them)
    if any(c in skip for c in range(CPB)) or any(
        s != skip for s in per_u_skip
    ):
        return _reference_np(q, k, v, att_mask, pad_mask)

    bf = ml_dtypes.bfloat16
    e4 = ml_dtypes.float8_e4m3

    qf = np.ascontiguousarray(
        q.reshape(U, L, D).transpose(0, 2, 1)
    ).astype(bf)
    kf = np.ascontiguousarray(
        k.reshape(U, L, D).transpose(0, 2, 1)
    ).astype(bf)

    # [V | 1] with padded keys zeroed
    vo = np.empty((U, L, D + 1), dtype=np.float32)
    vo[:, :, 0:D] = v.reshape(U, L, D)
    vo[:, :, D] = 1.0
    vo[pad_u] = 0.0
    voc = vo.reshape(U, NCH, KC, D + 1)             # [U, c, p, d]

    # fp8 paired weights: [U, p(=KC), pair, j, VO_W]
    vo8 = np.zeros((U, KC, NPAIR, 2, VO_W), dtype=np.float32)
    for pr in range(NPAIR):
        for j in (0, 1):
            c = 2 * pr + j
            if c < NCH and c not in skip:
                vo8[:, :, pr, j, 0:D + 1] = voc[:, c]
    vo8 = np.clip(vo8, -240.0, 240.0).astype(e4)

    # bf16 qb0 weights: [U, p, c(0..3), D+1]
    vo0 = np.ascontiguousarray(
        voc.transpose(0, 2, 1, 3)                   # [U, p, c, D+1]
    ).astype(bf)

    negi = (np.eye(KC, dtype=np.float32) * np.float32(-1e9)).astype(bf)
    u1 = (np.arange(KC)[None, :] < np.arange(KC)[:, None])
    utri = np.concatenate([u1, u1], axis=1).astype(bf)

    in_maps = []
    for c in range(N_CORES):
        sl = slice(c * NH, (c + 1) * NH)
        in_maps.append({
            "qt": qf[sl], "kt": kf[sl],
            "vo8": vo8[sl], "vo0": vo0[sl],
            "negi": negi, "utri": utri,
        })

    nc = _get_program(NH, L, D, skip)
    import os

    kwargs = {}
    if os.environ.get("BASS_KERNEL_PROFILE") == "1":
        kwargs = dict(trace=True, trace_cores=[0], stitch_traces=False)
    res = run_bass_kernel_spmd(nc, in_maps, list(range(N_CORES)), **kwargs)
    global LAST_RESULT
    LAST_RESULT = res
    raw = np.concatenate(
        [r["out"].astype(np.float32) for r in res.results], axis=0
    )
    # raw: [U, D+1, L] unnormalized -- normalize + transpose on host
    num = raw[:, 0:D, :]                            # [U, D, L]
    den = raw[:, D:D + 1, :]                        # [U, 1, L]
    out = (num / den).transpose(0, 2, 1)            # [U, L, D]
    out = np.ascontiguousarray(out).reshape(B, H, L, D)
    return out.astype(q.dtype, copy=False)


LAST_RESULT = None


# revision 34
# speedup vs baseline: 1.0031x; 1.0031x over previous
"""Causal attention with key padding for Trainium2, sharded over 8 NeuronCores.

Contract: kernel(**inputs) takes the FULL inputs (q, k, v, att_mask, pad_mask)
as numpy arrays and returns the FULL [B, H, L, D] output.

Strategy (v5):
  - Shard the 64 (batch, head) units across 8 cores; each core runs 8 heads.
  - Per head, key chunks are processed in adjacent PAIRS (2i, 2i+1):
    QK^T as two bf16 matmuls per (pair, query-block), one per 64-partition
    row-group half (contract D=64); the HW runs the pair concurrently
    (~213ns per pair of [128,512] score chunks).  Crossing (diagonal)
    chunks are PADDED to the full 512-query width -- dead columns are
    computed but never read, keeping every matmul pair-packable.
  - Causal boundary: a tiny extra matmul ((-1e9*I)^T @ strict_lower_ones)
    ACCUMULATES -1e9 onto the masked triangle of each crossing chunk's
    diagonal block in PSUM, so exp() kills it naturally in every path and
    no post-exp triangle multiply exists.
  - exp() splits across ScalarE (exact exp) and VectorE (Schraudolph via
    one tensor_scalar), cost-balanced at emission time:
      qb<3: bf16 probabilities (qb0 always ScalarE: short rows need exact
        exp); Schraudolph writes int16-bitcast-bf16.
      qb3: fp8e4 probabilities as p' = exp(s*scale - 3.5) (range fits
        fp8e4 for |s*scale| < 8.9); Schraudolph writes uint8-bitcast-fp8e4
        (HW rounds to nearest; saturates negatives to 0 = fp8 +0).
  - PV: for qb3 the pair's P~ and [V|1] are fp8e4 and ONE DoubleRow matmul
    contracts BOTH chunks (256 keys) in ~213ns -- 2x over bf16.  Dead/
    missing-member columns of the dedicated per-(pair,qb) fp8 slots are
    zeroed once at startup and persist across heads.  qb0..2 use bf16 PV
    per chunk (fp8 V noise fails the tolerance on concentrated rows
    there).  Padded keys have zeroed V rows and ones column, so padding
    costs nothing on device.
  - PSUM: 2 accumulator banks + 3 double-width score slots; qb2/qb3 PV
    starts are delayed (slot-granular software pipelining with per-qb
    FIFO release queues) so 2 accumulator banks always suffice.
  - [65, 512] accumulators (nums + denominator row) copy to SBUF (sc/ve
    balanced) and DMA out per head.  Normalization (num/den) and the
    final [D, L] -> [L, D] transpose happen on the host.
"""

import numpy as np

N_CORES = 8
KC = 128          # key-chunk (partition) size
QB = 512          # query-block width

_LOG2E = 1.4426950408889634
_EXP_C = 0.04305   # centers the log-linear interpolation error
_FP8_BIAS = 3.5    # p' = exp(s*scale - 3.5): keeps p' in fp8e4 range


# --------------------------------------------------------------------------
# numpy fallback (exact reference math) -- only used if the input masks do
# not match the causal + suffix-pad structure this kernel specializes to.
# --------------------------------------------------------------------------
def _reference_np(q, k, v, att_mask, pad_mask):
    B, H, L, D = q.shape
    scale = np.float32(1.0) / np.sqrt(np.float32(D))
    out = np.empty_like(q)
    for b in range(B):
        for h in range(H):
            att = (q[b, h] @ k[b, h].T) * scale
            att = att + att_mask[0, 0]
            att = np.where(pad_mask[b][None, :], -np.inf, att)
            att = att - att.max(axis=-1, keepdims=True)
            p = np.exp(att)
            p = p / p.sum(axis=-1, keepdims=True)
            out[b, h] = p @ v[b, h]
    return out


# --------------------------------------------------------------------------
# Bass program builder
# --------------------------------------------------------------------------
def _build_program(NH, L, D, skip):
    """Build the per-core SPMD Bass program.

    NH: heads per core.  L: sequence length.  D: head dim (must be 64).
    skip: frozenset of fully-padded key chunks (never computed).
    """
    import os

    import concourse.bacc as bacc
    import concourse.mybir as mybir
    import concourse.tile as tile

    f32 = mybir.dt.float32
    bf16 = mybir.dt.bfloat16
    fp8 = mybir.dt.float8e4
    u8 = mybir.dt.uint8
    i16 = mybir.dt.int16
    DR = mybir.MatmulPerfMode.DoubleRow

    NCH = L // KC          # 16 key chunks
    NQB = L // QB          # 4 query blocks
    CPB = QB // KC         # 4 chunks per query block
    NPAIR = (NCH + 1) // 2
    VO_W = 80              # fp8 weight stride (%16==0), cols 65..79 zero
    scale = float(1.0 / np.sqrt(np.float32(D)))
    exp_a16 = float(128.0 * _LOG2E * scale)
    exp_b16 = float(128.0 * (127.0 - _EXP_C))
    exp_a8 = float(8.0 * _LOG2E * scale)
    exp_b8 = float(8.0 * (7.0 - _EXP_C) - 8.0 * _FP8_BIAS * _LOG2E)

    ve_bias = float(os.environ.get("KVE_BIAS", "0"))
    n_warm = int(os.environ.get("KWARM", "28"))
    lag = int(os.environ.get("KLAG", "4"))
    qbmin8 = int(os.environ.get("KFP8_QBMIN", "3"))  # fp8 PV for qb >= this

    def fq(c):
        return c // CPB    # first query block needing chunk c

    def members_of(p):
        return [
            (j, 2 * p + j) for j in (0, 1)
            if 2 * p + j < NCH and (2 * p + j) not in skip
        ]

    def last_chunk(qb):
        cmax = min(CPB * qb + CPB - 1, NCH - 1)
        while cmax in skip:
            cmax -= 1
        return cmax

    def last_pair(qb):
        cmax = CPB * qb + CPB - 1
        for pp in range(min(cmax // 2, NPAIR - 1), -1, -1):
            if members_of(pp):
                return pp
        return 0

    nc = bacc.Bacc("TRN2", target_bir_lowering=False, debug=False)

    qt_d = nc.dram_tensor("qt", [NH, D, L], bf16, kind="ExternalInput")
    kt_d = nc.dram_tensor("kt", [NH, D, L], bf16, kind="ExternalInput")
    vo8_d = nc.dram_tensor("vo8", [NH, KC, NPAIR, 2, VO_W], fp8,
                           kind="ExternalInput")
    vo0_d = nc.dram_tensor("vo0", [NH, KC, NCH, D + 1], bf16,
                           kind="ExternalInput")
    negi_d = nc.dram_tensor("negi", [KC, KC], bf16, kind="ExternalInput")
    utri_d = nc.dram_tensor("utri", [KC, 2 * KC], bf16, kind="ExternalInput")
    out_d = nc.dram_tensor("out", [NH, D + 1, L], bf16, kind="ExternalOutput")

    load = {"sc": 0.0, "ve": 0.0}

    def route(cost_sc, cost_ve):
        if load["sc"] + cost_sc <= load["ve"] + cost_ve + ve_bias:
            load["sc"] += cost_sc
            return "sc"
        load["ve"] += cost_ve
        return "ve"

    with tile.TileContext(nc) as tc:
        with (
            tc.tile_pool(name="consts", bufs=1) as consts,
            tc.tile_pool(name="ktp", bufs=2) as ktp,
            tc.tile_pool(name="qtp", bufs=2) as qtp,
            tc.tile_pool(name="vo8p", bufs=2) as vo8p,
            tc.tile_pool(name="vo0p", bufs=2) as vo0p,
            tc.tile_pool(name="pt8c", bufs=1) as pt8c,
            tc.tile_pool(name="pt0p", bufs=12) as pt0p,
            tc.tile_pool(name="osb", bufs=2) as osb,
            tc.tile_pool(name="stg", bufs=3, space="PSUM") as stgp,
            tc.tile_pool(name="acc", bufs=2, space="PSUM") as accp,
        ):
            negi = consts.tile([KC, KC], bf16, tag="negi")
            utri = consts.tile([KC, 2 * KC], bf16, tag="utri")
            bias_m2 = consts.tile([KC, 1], f32, tag="bias")
            nc.vector.memset(bias_m2[:], -_FP8_BIAS)

            # Warm-up with no DMA dependency: sustained PE activity makes
            # the clock gate grant full rate sooner; a dummy exp pulls the
            # ACT table load off the critical path -- all while the first
            # head's tensors stream in.
            wsrc = consts.tile([KC, QB], bf16, tag="wsrc")
            wout = consts.tile([KC, 1], f32, tag="wout")
            nc.vector.memset(wsrc[:], 0.0)
            warm = stgp.tile([KC, 2 * QB], f32, tag="stg")
            for i in range(n_warm):
                nc.tensor.matmul(
                    out=warm[:, 0:QB], lhsT=wsrc[:, 0:KC], rhs=wsrc[:],
                    start=True, stop=True,
                )
                if i == 0:
                    nc.scalar.activation(
                        out=wout[:], in_=warm[:, 0:1],
                        func=mybir.ActivationFunctionType.Exp,
                    )

            nc.sync.dma_start(out=negi[:], in_=negi_d[:])
            nc.sync.dma_start(out=utri[:], in_=utri_d[:])

            # Dedicated fp8 probability slots per (pair, qb>=1).  Dead
            # columns (crossing-chunk heads, missing pair members) are
            # zeroed ONCE here; exp rewrites exactly the live columns each
            # head, so the zeros persist.
            pt8_slot = {}
            for p in range(NPAIR):
                mem = members_of(p)
                if not mem:
                    continue
                for qb in range(max(qbmin8, fq(2 * p)), NQB):
                    t = pt8c.tile([KC, 2, QB], fp8, tag=f"pt8_{p}_{qb}")
                    pt8_slot[(p, qb)] = t
                    live_j = {j for j, _ in mem}
                    for j in (0, 1):
                        if j not in live_j:
                            nc.gpsimd.memset(t[:, j, :], 0.0)
                            continue
                        c = 2 * p + j
                        dead = max(0, c * KC - qb * QB)
                        if dead > 0:
                            nc.gpsimd.memset(t[:, j, 0:dead], 0.0)

            # ------------------------------------------------------------
            # software-pipelined emission
            # ------------------------------------------------------------
            pv_queue = []    # (release_tick, qb, kind, payload)
            epi_queue = []   # (release_tick, h, qb, acc_of, o_t)
            tick = [0]
            # qb2/qb3 PV starts are delayed so only 2 accumulator banks are
            # ever live (their dedicated pt tiles hold the probabilities);
            # ticks count SLOTS (one (pair, qb) item each)
            pv_extra = {0: 0, 1: 0, 2: 6, 3: 12}

            def emit_pv(job):
                _, qb, kind, payload = job
                if kind == "fp8":
                    p, acc_of, vo8_t, start, stop = payload
                    if qb not in acc_of:
                        acc_of[qb] = accp.tile([D + 1, QB], f32,
                                               name="acc", tag="acc")
                    nc.tensor.matmul(
                        out=acc_of[qb][:],
                        lhsT=vo8_t[:, p, :, 0:D + 1],
                        rhs=pt8_slot[(p, qb)][:, :, :],
                        start=start, stop=stop,
                        perf_mode=DR,
                    )
                else:
                    c, acc_of, vo0_t, pt0, j, dead, start, stop = payload
                    if qb not in acc_of:
                        acc_of[qb] = accp.tile([D + 1, QB], f32,
                                               name="acc", tag="acc")
                    nc.tensor.matmul(
                        out=acc_of[qb][:, dead:QB],
                        lhsT=vo0_t[:, c, :],
                        rhs=pt0[:, j * QB + dead:(j + 1) * QB],
                        start=start, stop=stop,
                    )

            def emit_epi(job):
                _, h, qb, acc_of, o_t = job
                c_sc = (394 + QB) / 1.2
                c_ve = (196 + QB) / 0.96
                dst = o_t[:, qb * QB:(qb + 1) * QB]
                if route(c_sc, c_ve) == "sc":
                    nc.scalar.copy(out=dst, in_=acc_of[qb][:])
                else:
                    nc.vector.tensor_copy(out=dst, in_=acc_of[qb][:])
                if qb == NQB - 1:
                    # last block of head h: ship the whole [65, L] tile out
                    nc.gpsimd.dma_start(out=out_d[h], in_=o_t[:])

            def flush(drain=False):
                # epilogues first: the acc bank is recycled by a later PV
                # allocation, whose emission must come after the copy
                def pop_epis():
                    while epi_queue and (drain or epi_queue[0][0] <= tick[0]):
                        emit_epi(epi_queue.pop(0))

                def pop_pvs():
                    # per-qb FIFO: a qb's job may only run once all earlier
                    # jobs of the same qb have been emitted; emit eligible
                    # jobs round-robin across qbs so consecutive PV matmuls
                    # hit different accumulator banks
                    blocked = set()
                    eligible = []
                    i = 0
                    while i < len(pv_queue):
                        job = pv_queue[i]
                        if job[1] not in blocked and (
                            drain or job[0] <= tick[0] - lag
                        ):
                            eligible.append(pv_queue.pop(i))
                        else:
                            blocked.add(job[1])
                            i += 1
                    byqb = {}
                    for job in eligible:
                        byqb.setdefault(job[1], []).append(job)
                    order = sorted(byqb)
                    while byqb:
                        for qb_ in list(order):
                            if qb_ in byqb:
                                emit_pv(byqb[qb_].pop(0))
                                if not byqb[qb_]:
                                    del byqb[qb_]

                if drain:
                    pop_pvs()
                    pop_epis()
                else:
                    pop_epis()
                    pop_pvs()

            for h in range(NH):
                kt_t = ktp.tile([KC, L], bf16)
                qt_t = qtp.tile([KC, L], bf16)
                vo8_t = vo8p.tile([KC, NPAIR, 2, VO_W], fp8)
                vo0_t = vo0p.tile([KC, NCH, D + 1], bf16)
                col_splits = ((0, QB), (QB, L)) if h == 0 else ((0, L),)
                for lo, hi in col_splits:
                    for half in (0, 1):
                        nc.sync.dma_start(
                            out=kt_t[half * D:(half + 1) * D, lo:hi],
                            in_=kt_d[h, :, lo:hi],
                        )
                        nc.sync.dma_start(
                            out=qt_t[half * D:(half + 1) * D, lo:hi],
                            in_=qt_d[h, :, lo:hi],
                        )
                nc.sync.dma_start(out=vo8_t[:], in_=vo8_d[h])
                nc.sync.dma_start(out=vo0_t[:], in_=vo0_d[h])
                o_t = osb.tile([D + 1, L], bf16)

                acc_of = {}
                qb_started = set()
                for p in range(NPAIR):
                    mem = members_of(p)
                    if not mem:
                        continue
                    f = fq(2 * p)
                    for qb in range(f, NQB):
                        stg = stgp.tile([KC, 2 * QB], f32, tag="stg")
                        for j, c in mem:
                            crossing = c * KC - qb * QB >= 0
                            nc.tensor.matmul(
                                out=stg[:, j * QB:(j + 1) * QB],
                                lhsT=kt_t[j * D:(j + 1) * D,
                                          c * KC:(c + 1) * KC],
                                rhs=qt_t[j * D:(j + 1) * D,
                                         qb * QB:(qb + 1) * QB],
                                start=True, stop=not crossing,
                            )
                        # causal boundary: accumulate -1e9 onto the
                        # strictly-upper triangle of each diagonal block
                        for j, c in mem:
                            dead = c * KC - qb * QB
                            if dead >= 0:
                                nc.tensor.matmul(
                                    out=stg[:, j * QB + dead:
                                            j * QB + dead + KC],
                                    lhsT=negi[:],
                                    rhs=utri[:, 0:KC],
                                    start=False, stop=True,
                                )
                        # exp: contiguous runs in the flat [0, 2*QB) space
                        runs = []
                        for j, c in mem:
                            dead = max(0, c * KC - qb * QB)
                            r0, r1 = j * QB + dead, (j + 1) * QB
                            if runs and runs[-1][1] == r0:
                                runs[-1][1] = r1
                            else:
                                runs.append([r0, r1])
                        if qb < qbmin8:
                            # bf16 path: one [KC, 2*QB] pt tile per pair
                            pt0 = pt0p.tile([KC, 2 * QB], bf16)
                            for r0, r1 in runs:
                                w = r1 - r0
                                c_sc = (394 + w) / 1.2
                                c_ve = (196 + w) / 0.96
                                if qb == 0:
                                    load["sc"] += c_sc
                                    eng = "sc"
                                else:
                                    eng = route(c_sc, c_ve)
                                if eng == "sc":
                                    nc.scalar.activation(
                                        out=pt0[:, r0:r1], in_=stg[:, r0:r1],
                                        func=mybir.ActivationFunctionType.Exp,
                                        scale=scale,
                                    )
                                else:
                                    nc.vector.tensor_scalar(
                                        out=pt0[:, r0:r1].bitcast(i16),
                                        in0=stg[:, r0:r1],
                                        scalar1=exp_a16, scalar2=exp_b16,
                                        op0=mybir.AluOpType.mult,
                                        op1=mybir.AluOpType.add,
                                    )
                            for j, c in mem:
                                dead = max(0, c * KC - qb * QB)
                                extra = (0 if qb in qb_started
                                         else pv_extra.get(qb, 0))
                                qb_started.add(qb)
                                pv_queue.append((
                                    tick[0] + extra, qb, "bf16",
                                    (c, acc_of, vo0_t, pt0, j, dead,
                                     c == 0, c == last_chunk(qb)),
                                ))
                        else:
                            pt8 = pt8_slot[(p, qb)]
                            pt8f = pt8.rearrange("p two q -> p (two q)")
                            for r0, r1 in runs:
                                w = r1 - r0
                                c_sc = (394 + w) / 1.2
                                c_ve = (196 + w) / 0.96
                                if route(c_sc, c_ve) == "sc":
                                    nc.scalar.activation(
                                        out=pt8f[:, r0:r1],
                                        in_=stg[:, r0:r1],
                                        func=mybir.ActivationFunctionType.Exp,
                                        scale=scale, bias=bias_m2[:],
                                    )
                                else:
                                    nc.vector.tensor_scalar(
                                        out=pt8f[:, r0:r1].bitcast(u8),
                                        in0=stg[:, r0:r1],
                                        scalar1=exp_a8, scalar2=exp_b8,
                                        op0=mybir.AluOpType.mult,
                                        op1=mybir.AluOpType.add,
                                    )
                            extra = (0 if qb in qb_started
                                     else pv_extra.get(qb, 0))
                            qb_started.add(qb)
                            pv_queue.append((
                                tick[0] + extra, qb, "fp8",
                                (p, acc_of, vo8_t,
                                 p == 0, p == last_pair(qb)),
                            ))
                        if p == last_pair(qb):
                            epi_queue.append((
                                tick[0] + lag + 3,
                                h, qb, acc_of, o_t,
                            ))
                        tick[0] += 1
                        flush()

            flush(drain=True)
            flush(drain=True)
    if os.environ.get("KDEBUG_ROUTE"):
        print(f"route loads: sc={load['sc']:.0f}ns ve={load['ve']:.0f}ns "
              f"(ve_bias={ve_bias:.0f})")
    nc.finalize()
    return nc


# --------------------------------------------------------------------------
# host-side wrapper
# --------------------------------------------------------------------------
_PROG_CACHE = {}


def _get_program(NH, L, D, skip):
    key = (NH, L, D, skip)
    if key not in _PROG_CACHE:
        _PROG_CACHE[key] = _build_program(NH, L, D, skip)
    return _PROG_CACHE[key]


def _causal_ok(att_mask, L):
    if att_mask.shape != (1, 1, L, L):
        return False
    m = att_mask[0, 0]
    iu = np.triu_indices(L, 1)
    if not np.all(m[iu] == np.float32(-1e9)):
        return False
    il = np.tril_indices(L)
    return bool(np.all(m[il] == 0.0))


def kernel(q, k, v, att_mask, pad_mask):
    import ml_dtypes

    from concourse.bass_utils import run_bass_kernel_spmd

    B, H, L, D = q.shape
    U = B * H
    NCH = L // KC
    CPB = QB // KC
    NPAIR = (NCH + 1) // 2
    VO_W = 80
    if (
        U % N_CORES != 0
        or L % QB != 0
        or D != 64
        or not _causal_ok(att_mask, L)
    ):
        return _reference_np(q, k, v, att_mask, pad_mask)

    NH = U // N_CORES  # units (heads) per core

    pad = np.asarray(pad_mask, dtype=bool)          # [B, L]
    pad_u = np.repeat(pad, H, axis=0)               # [U, L]

    skip = frozenset(
        kc for kc in range(NCH)
        if np.all(pad_u[:, kc * KC:(kc + 1) * KC])
    )
    per_u_skip = [
        frozenset(
            kc for kc in range(NCH)
            if np.all(pad_u[u, kc * KC:(kc + 1) * KC])
        )
        for u in range(U)
    ]
    # chunks 0..CPB-1 must exist (qb0 bf16 path assumes them)
    if any(c in skip for c in range(CPB)) or any(
        s != skip for s in per_u_skip
    ):
        return _reference_np(q, k, v, att_mask, pad_mask)

    bf = ml_dtypes.bfloat16
    e4 = ml_dtypes.float8_e4m3

    qf = np.ascontiguousarray(
        q.reshape(U, L, D).transpose(0, 2, 1)
    ).astype(bf)
    kf = np.ascontiguousarray(
        k.reshape(U, L, D).transpose(0, 2, 1)
    ).astype(bf)

    # [V | 1] with padded keys zeroed
    vo = np.empty((U, L, D + 1), dtype=np.float32)
    vo[:, :, 0:D] = v.reshape(U, L, D)
    vo[:, :, D] = 1.0
    vo[pad_u] = 0.0
    voc = vo.reshape(U, NCH, KC, D + 1)             # [U, c, p, d]

    # fp8 paired weights: [U, p(=KC), pair, j, VO_W]
    vo8 = np.zeros((U, KC, NPAIR, 2, VO_W), dtype=np.float32)
    for pr in range(NPAIR):
        for j in (0, 1):
            c = 2 * pr + j
            if c < NCH and c not in skip:
                vo8[:, :, pr, j, 0:D + 1] = voc[:, c]
    vo8 = np.clip(vo8, -240.0, 240.0).astype(e4)

    # bf16 qb0 weights: [U, p, c(0..3), D+1]
    vo0 = np.ascontiguousarray(
        voc.transpose(0, 2, 1, 3)                   # [U, p, c, D+1]
    ).astype(bf)

    negi = (np.eye(KC, dtype=np.float32) * np.float32(-1e9)).astype(bf)
    u1 = (np.arange(KC)[None, :] < np.arange(KC)[:, None])
    utri = np.concatenate([u1, u1], axis=1).astype(bf)

    in_maps = []
    for c in range(N_CORES):
        sl = slice(c * NH, (c + 1) * NH)
        in_maps.append({
            "qt": qf[sl], "kt": kf[sl],
            "vo8": vo8[sl], "vo0": vo0[sl],
            "negi": negi, "utri": utri,
        })

    nc = _get_program(NH, L, D, skip)
    import os

    kwargs = {}
    if os.environ.get("BASS_KERNEL_PROFILE") == "1":
        kwargs = dict(trace=True, trace_cores=[0], stitch_traces=False)
    res = run_bass_kernel_spmd(nc, in_maps, list(range(N_CORES)), **kwargs)
    global LAST_RESULT
    LAST_RESULT = res
    raw = np.concatenate(
        [r["out"].astype(np.float32) for r in res.results], axis=0
    )
    # raw: [U, D+1, L] unnormalized -- normalize + transpose on host
    num = raw[:, 0:D, :]                            # [U, D, L]
    den = raw[:, D:D + 1, :]                        # [U, 1, L]
    out = (num / den).transpose(0, 2, 1)            # [U, L, D]
    out = np.ascontiguousarray(out).reshape(B, H, L, D)
    return out.astype(q.dtype, copy=False)


LAST_RESULT = None


# revision 35
# speedup vs baseline: 1.0075x; 1.0044x over previous
"""Causal attention with key padding for Trainium2, sharded over 8 NeuronCores.

Contract: kernel(**inputs) takes the FULL inputs (q, k, v, att_mask, pad_mask)
as numpy arrays and returns the FULL [B, H, L, D] output.

Strategy (v5):
  - Shard the 64 (batch, head) units across 8 cores; each core runs 8 heads.
  - Per head, key chunks are processed in adjacent PAIRS (2i, 2i+1):
    QK^T as two bf16 matmuls per (pair, query-block), one per 64-partition
    row-group half (contract D=64); the HW runs the pair concurrently
    (~213ns per pair of [128,512] score chunks).  Crossing (diagonal)
    chunks are PADDED to the full 512-query width -- dead columns are
    computed but never read, keeping every matmul pair-packable.
  - Causal boundary: a tiny extra matmul ((-1e9*I)^T @ strict_lower_ones)
    ACCUMULATES -1e9 onto the masked triangle of each crossing chunk's
    diagonal block in PSUM, so exp() kills it naturally in every path and
    no post-exp triangle multiply exists.
  - exp() splits across ScalarE (exact exp) and VectorE (Schraudolph via
    one tensor_scalar), cost-balanced at emission time:
      qb<3: bf16 probabilities (qb0 always ScalarE: short rows need exact
        exp); Schraudolph writes int16-bitcast-bf16.
      qb3: fp8e4 probabilities as p' = exp(s*scale - 3.5) (range fits
        fp8e4 for |s*scale| < 8.9); Schraudolph writes uint8-bitcast-fp8e4
        (HW rounds to nearest; saturates negatives to 0 = fp8 +0).
  - PV: for qb3 the pair's P~ and [V|1] are fp8e4 and ONE DoubleRow matmul
    contracts BOTH chunks (256 keys) in ~213ns -- 2x over bf16.  Dead/
    missing-member columns of the dedicated per-(pair,qb) fp8 slots are
    zeroed once at startup and persist across heads.  qb0..2 use bf16 PV
    per chunk (fp8 V noise fails the tolerance on concentrated rows
    there).  Padded keys have zeroed V rows and ones column, so padding
    costs nothing on device.
  - PSUM: 2 accumulator banks + 3 double-width score slots; qb2/qb3 PV
    starts are delayed (slot-granular software pipelining with per-qb
    FIFO release queues) so 2 accumulator banks always suffice.
  - [65, 512] accumulators (nums + denominator row) copy to SBUF (sc/ve
    balanced) and DMA out per head.  Normalization (num/den) and the
    final [D, L] -> [L, D] transpose happen on the host.
"""

import numpy as np

N_CORES = 8
KC = 128          # key-chunk (partition) size
QB = 512          # query-block width

_LOG2E = 1.4426950408889634
_EXP_C = 0.04305   # centers the log-linear interpolation error
_FP8_BIAS = 3.5    # p' = exp(s*scale - 3.5): keeps p' in fp8e4 range


# --------------------------------------------------------------------------
# numpy fallback (exact reference math) -- only used if the input masks do
# not match the causal + suffix-pad structure this kernel specializes to.
# --------------------------------------------------------------------------
def _reference_np(q, k, v, att_mask, pad_mask):
    B, H, L, D = q.shape
    scale = np.float32(1.0) / np.sqrt(np.float32(D))
    out = np.empty_like(q)
    for b in range(B):
        for h in range(H):
            att = (q[b, h] @ k[b, h].T) * scale
            att = att + att_mask[0, 0]
            att = np.where(pad_mask[b][None, :], -np.inf, att)
            att = att - att.max(axis=-1, keepdims=True)
            p = np.exp(att)
            p = p / p.sum(axis=-1, keepdims=True)
            out[b, h] = p @ v[b, h]
    return out


# --------------------------------------------------------------------------
# Bass program builder
# --------------------------------------------------------------------------
def _build_program(NH, L, D, skip):
    """Build the per-core SPMD Bass program.

    NH: heads per core.  L: sequence length.  D: head dim (must be 64).
    skip: frozenset of fully-padded key chunks (never computed).
    """
    import os

    import concourse.bacc as bacc
    import concourse.mybir as mybir
    import concourse.tile as tile

    f32 = mybir.dt.float32
    bf16 = mybir.dt.bfloat16
    fp8 = mybir.dt.float8e4
    u8 = mybir.dt.uint8
    i16 = mybir.dt.int16
    DR = mybir.MatmulPerfMode.DoubleRow

    NCH = L // KC          # 16 key chunks
    NQB = L // QB          # 4 query blocks
    CPB = QB // KC         # 4 chunks per query block
    NPAIR = (NCH + 1) // 2
    VO_W = 80              # fp8 weight stride (%16==0), cols 65..79 zero
    scale = float(1.0 / np.sqrt(np.float32(D)))
    exp_a16 = float(128.0 * _LOG2E * scale)
    exp_b16 = float(128.0 * (127.0 - _EXP_C))
    exp_a8 = float(8.0 * _LOG2E * scale)
    exp_b8 = float(8.0 * (7.0 - _EXP_C) - 8.0 * _FP8_BIAS * _LOG2E)

    ve_bias = float(os.environ.get("KVE_BIAS", "0"))
    n_warm = int(os.environ.get("KWARM", "28"))
    lag = int(os.environ.get("KLAG", "4"))
    qbmin8 = int(os.environ.get("KFP8_QBMIN", "3"))  # fp8 PV for qb >= this

    def fq(c):
        return c // CPB    # first query block needing chunk c

    def members_of(p):
        return [
            (j, 2 * p + j) for j in (0, 1)
            if 2 * p + j < NCH and (2 * p + j) not in skip
        ]

    def last_chunk(qb):
        cmax = min(CPB * qb + CPB - 1, NCH - 1)
        while cmax in skip:
            cmax -= 1
        return cmax

    def last_pair(qb):
        cmax = CPB * qb + CPB - 1
        for pp in range(min(cmax // 2, NPAIR - 1), -1, -1):
            if members_of(pp):
                return pp
        return 0

    nc = bacc.Bacc("TRN2", target_bir_lowering=False, debug=False)

    qt_d = nc.dram_tensor("qt", [NH, D, L], bf16, kind="ExternalInput")
    kt_d = nc.dram_tensor("kt", [NH, D, L], bf16, kind="ExternalInput")
    vo8_d = nc.dram_tensor("vo8", [NH, KC, NPAIR, 2, VO_W], fp8,
                           kind="ExternalInput")
    vo0_d = nc.dram_tensor("vo0", [NH, KC, NCH, D + 1], bf16,
                           kind="ExternalInput")
    negi_d = nc.dram_tensor("negi", [KC, KC], bf16, kind="ExternalInput")
    utri_d = nc.dram_tensor("utri", [KC, 2 * KC], bf16, kind="ExternalInput")
    out_d = nc.dram_tensor("out", [NH, D + 1, L], bf16, kind="ExternalOutput")

    load = {"sc": 0.0, "ve": 0.0}

    def route(cost_sc, cost_ve):
        if load["sc"] + cost_sc <= load["ve"] + cost_ve + ve_bias:
            load["sc"] += cost_sc
            return "sc"
        load["ve"] += cost_ve
        return "ve"

    with tile.TileContext(nc) as tc:
        with (
            tc.tile_pool(name="consts", bufs=1) as consts,
            tc.tile_pool(name="ktp", bufs=2) as ktp,
            tc.tile_pool(name="qtp", bufs=2) as qtp,
            tc.tile_pool(name="vo8p", bufs=2) as vo8p,
            tc.tile_pool(name="vo0p", bufs=2) as vo0p,
            tc.tile_pool(name="pt8c", bufs=1) as pt8c,
            tc.tile_pool(name="pt0p", bufs=12) as pt0p,
            tc.tile_pool(name="osb", bufs=2) as osb,
            tc.tile_pool(name="stg", bufs=3, space="PSUM") as stgp,
            tc.tile_pool(name="acc", bufs=2, space="PSUM") as accp,
        ):
            negi = consts.tile([KC, KC], bf16, tag="negi")
            utri = consts.tile([KC, 2 * KC], bf16, tag="utri")
            bias_m2 = consts.tile([KC, 1], f32, tag="bias")
            nc.vector.memset(bias_m2[:], -_FP8_BIAS)

            # Warm-up with no DMA dependency: sustained PE activity makes
            # the clock gate grant full rate sooner; a dummy exp pulls the
            # ACT table load off the critical path -- all while the first
            # head's tensors stream in.
            wsrc = consts.tile([KC, QB], bf16, tag="wsrc")
            wout = consts.tile([KC, 1], f32, tag="wout")
            nc.vector.memset(wsrc[:], 0.0)
            warm = stgp.tile([KC, 2 * QB], f32, tag="stg")
            for i in range(n_warm):
                nc.tensor.matmul(
                    out=warm[:, 0:QB], lhsT=wsrc[:, 0:KC], rhs=wsrc[:],
                    start=True, stop=True,
                )
                if i == 0:
                    nc.scalar.activation(
                        out=wout[:], in_=warm[:, 0:1],
                        func=mybir.ActivationFunctionType.Exp,
                    )

            nc.sync.dma_start(out=negi[:], in_=negi_d[:])
            nc.sync.dma_start(out=utri[:], in_=utri_d[:])

            # Dedicated fp8 probability slots per (pair, qb>=1).  Dead
            # columns (crossing-chunk heads, missing pair members) are
            # zeroed ONCE here; exp rewrites exactly the live columns each
            # head, so the zeros persist.
            pt8_slot = {}
            for p in range(NPAIR):
                mem = members_of(p)
                if not mem:
                    continue
                for qb in range(max(qbmin8, fq(2 * p)), NQB):
                    t = pt8c.tile([KC, 2, QB], fp8, tag=f"pt8_{p}_{qb}")
                    pt8_slot[(p, qb)] = t
                    live_j = {j for j, _ in mem}
                    for j in (0, 1):
                        if j not in live_j:
                            nc.gpsimd.memset(t[:, j, :], 0.0)
                            continue
                        c = 2 * p + j
                        dead = max(0, c * KC - qb * QB)
                        if dead > 0:
                            nc.gpsimd.memset(t[:, j, 0:dead], 0.0)

            # ------------------------------------------------------------
            # software-pipelined emission
            # ------------------------------------------------------------
            pv_queue = []    # (release_tick, qb, kind, payload)
            epi_queue = []   # (release_tick, h, qb, acc_of, o_t)
            tick = [0]
            # qb2/qb3 PV starts are delayed so only 2 accumulator banks are
            # ever live (their dedicated pt tiles hold the probabilities);
            # ticks count SLOTS (one (pair, qb) item each)
            pv_extra = {0: 0, 1: 0, 2: 6, 3: 12}

            def emit_pv(job):
                _, qb, kind, payload = job
                if kind == "fp8":
                    p, acc_of, vo8_t, start, stop = payload
                    if qb not in acc_of:
                        acc_of[qb] = accp.tile([D + 1, QB], f32,
                                               name="acc", tag="acc")
                    nc.tensor.matmul(
                        out=acc_of[qb][:],
                        lhsT=vo8_t[:, p, :, 0:D + 1],
                        rhs=pt8_slot[(p, qb)][:, :, :],
                        start=start, stop=stop,
                        perf_mode=DR,
                    )
                else:
                    c, acc_of, vo0_t, pt0, j, dead, start, stop = payload
                    if qb not in acc_of:
                        acc_of[qb] = accp.tile([D + 1, QB], f32,
                                               name="acc", tag="acc")
                    nc.tensor.matmul(
                        out=acc_of[qb][:, dead:QB],
                        lhsT=vo0_t[:, c, :],
                        rhs=pt0[:, j * QB + dead:(j + 1) * QB],
                        start=start, stop=stop,
                    )

            def emit_epi(job):
                _, h, qb, acc_of, o_t = job
                c_sc = (394 + QB) / 1.2
                c_ve = (196 + QB) / 0.96
                dst = o_t[:, qb * QB:(qb + 1) * QB]
                if route(c_sc, c_ve) == "sc":
                    nc.scalar.copy(out=dst, in_=acc_of[qb][:])
                else:
                    nc.vector.tensor_copy(out=dst, in_=acc_of[qb][:])
                if qb == NQB - 1:
                    # last block of head h: ship the whole [65, L] tile out
                    nc.gpsimd.dma_start(out=out_d[h], in_=o_t[:])

            def flush(drain=False):
                # epilogues first: the acc bank is recycled by a later PV
                # allocation, whose emission must come after the copy
                def pop_epis():
                    while epi_queue and (drain or epi_queue[0][0] <= tick[0]):
                        emit_epi(epi_queue.pop(0))

                def pop_pvs():
                    # per-qb FIFO: a qb's job may only run once all earlier
                    # jobs of the same qb have been emitted; emit eligible
                    # jobs round-robin across qbs so consecutive PV matmuls
                    # hit different accumulator banks
                    blocked = set()
                    eligible = []
                    i = 0
                    while i < len(pv_queue):
                        job = pv_queue[i]
                        if job[1] not in blocked and (
                            drain or job[0] <= tick[0] - lag
                        ):
                            eligible.append(pv_queue.pop(i))
                        else:
                            blocked.add(job[1])
                            i += 1
                    byqb = {}
                    for job in eligible:
                        byqb.setdefault(job[1], []).append(job)
                    order = sorted(byqb)
                    while byqb:
                        for qb_ in list(order):
                            if qb_ in byqb:
                                emit_pv(byqb[qb_].pop(0))
                                if not byqb[qb_]:
                                    del byqb[qb_]

                if drain:
                    pop_pvs()
                    pop_epis()
                else:
                    pop_epis()
                    pop_pvs()

            for h in range(NH):
                kt_t = ktp.tile([KC, L], bf16)
                qt_t = qtp.tile([KC, L], bf16)
                vo8_t = vo8p.tile([KC, NPAIR, 2, VO_W], fp8)
                vo0_t = vo0p.tile([KC, NCH, D + 1], bf16)
                col_splits = ((0, QB), (QB, L)) if h == 0 else ((0, L),)
                for lo, hi in col_splits:
                    for half in (0, 1):
                        nc.sync.dma_start(
                            out=kt_t[half * D:(half + 1) * D, lo:hi],
                            in_=kt_d[h, :, lo:hi],
                        )
                        nc.sync.dma_start(
                            out=qt_t[half * D:(half + 1) * D, lo:hi],
                            in_=qt_d[h, :, lo:hi],
                        )
                nc.sync.dma_start(out=vo8_t[:], in_=vo8_d[h])
                nc.sync.dma_start(out=vo0_t[:], in_=vo0_d[h])
                o_t = osb.tile([D + 1, L], bf16)

                acc_of = {}
                qb_started = set()
                for p in range(NPAIR):
                    mem = members_of(p)
                    if not mem:
                        continue
                    f = fq(2 * p)
                    for qb in range(f, NQB):
                        stg = stgp.tile([KC, 2 * QB], f32, tag="stg")
                        for j, c in mem:
                            crossing = c * KC - qb * QB >= 0
                            nc.tensor.matmul(
                                out=stg[:, j * QB:(j + 1) * QB],
                                lhsT=kt_t[j * D:(j + 1) * D,
                                          c * KC:(c + 1) * KC],
                                rhs=qt_t[j * D:(j + 1) * D,
                                         qb * QB:(qb + 1) * QB],
                                start=True, stop=not crossing,
                            )
                        # causal boundary: accumulate -1e9 onto the
                        # strictly-upper triangle of each diagonal block
                        for j, c in mem:
                            dead = c * KC - qb * QB
                            if dead >= 0:
                                nc.tensor.matmul(
                                    out=stg[:, j * QB + dead:
                                            j * QB + dead + KC],
                                    lhsT=negi[:],
                                    rhs=utri[:, 0:KC],
                                    start=False, stop=True,
                                )
                        # exp: contiguous runs in the flat [0, 2*QB) space
                        runs = []
                        for j, c in mem:
                            dead = max(0, c * KC - qb * QB)
                            r0, r1 = j * QB + dead, (j + 1) * QB
                            if runs and runs[-1][1] == r0:
                                runs[-1][1] = r1
                            else:
                                runs.append([r0, r1])
                        if qb < qbmin8:
                            # bf16 path: one [KC, 2*QB] pt tile per pair
                            pt0 = pt0p.tile([KC, 2 * QB], bf16)
                            for r0, r1 in runs:
                                w = r1 - r0
                                c_sc = (394 + w) / 1.2
                                c_ve = (196 + w) / 0.96
                                if qb == 0:
                                    load["sc"] += c_sc
                                    eng = "sc"
                                else:
                                    eng = route(c_sc, c_ve)
                                if eng == "sc":
                                    nc.scalar.activation(
                                        out=pt0[:, r0:r1], in_=stg[:, r0:r1],
                                        func=mybir.ActivationFunctionType.Exp,
                                        scale=scale,
                                    )
                                else:
                                    nc.vector.tensor_scalar(
                                        out=pt0[:, r0:r1].bitcast(i16),
                                        in0=stg[:, r0:r1],
                                        scalar1=exp_a16, scalar2=exp_b16,
                                        op0=mybir.AluOpType.mult,
                                        op1=mybir.AluOpType.add,
                                    )
                            for j, c in mem:
                                dead = max(0, c * KC - qb * QB)
                                extra = (0 if qb in qb_started
                                         else pv_extra.get(qb, 0))
                                qb_started.add(qb)
                                pv_queue.append((
                                    tick[0] + extra, qb, "bf16",
                                    (c, acc_of, vo0_t, pt0, j, dead,
                                     c == 0, c == last_chunk(qb)),
                                ))
                        else:
                            pt8 = pt8_slot[(p, qb)]
                            pt8f = pt8.rearrange("p two q -> p (two q)")
                            for r0, r1 in runs:
                                w = r1 - r0
                                c_sc = (394 + w) / 1.2
                                c_ve = (196 + w) / 0.96
                                if route(c_sc, c_ve) == "sc":
                                    nc.scalar.activation(
                                        out=pt8f[:, r0:r1],
                                        in_=stg[:, r0:r1],
                                        func=mybir.ActivationFunctionType.Exp,
                                        scale=scale, bias=bias_m2[:],
                                    )
                                else:
                                    nc.vector.tensor_scalar(
                                        out=pt8f[:, r0:r1].bitcast(u8),
                                        in0=stg[:, r0:r1],
                                        scalar1=exp_a8, scalar2=exp_b8,
                                        op0=mybir.AluOpType.mult,
                                        op1=mybir.AluOpType.add,
                                    )
                            extra = (0 if qb in qb_started
                                     else pv_extra.get(qb, 0))
                            qb_started.add(qb)
                            pv_queue.append((
                                tick[0] + extra, qb, "fp8",
                                (p, acc_of, vo8_t,
                                 p == 0, p == last_pair(qb)),
                            ))
                        if p == last_pair(qb):
                            epi_queue.append((
                                tick[0] + lag + 1,
                                h, qb, acc_of, o_t,
                            ))
                        tick[0] += 1
                        flush()

            flush(drain=True)
            flush(drain=True)
    if os.environ.get("KDEBUG_ROUTE"):
        print(f"route loads: sc={load['sc']:.0f}ns ve={load['ve']:.0f}ns "
              f"(ve_bias={ve_bias:.0f})")
    nc.finalize()
    return nc


# --------------------------------------------------------------------------
# host-side wrapper
# --------------------------------------------------------------------------
_PROG_CACHE = {}


def _get_program(NH, L, D, skip):
    key = (NH, L, D, skip)
    if key not in _PROG_CACHE:
        _PROG_CACHE[key] = _build_program(NH, L, D, skip)
    return _PROG_CACHE[key]


def _causal_ok(att_mask, L):
    if att_mask.shape != (1, 1, L, L):
        return False
    m = att_mask[0, 0]
    iu = np.triu_indices(L, 1)
    if not np.all(m[iu] == np.float32(-1e9)):
        return False
    il = np.tril_indices(L)
    return bool(np.all(m[il] == 0.0))


def kernel(q, k, v, att_mask, pad_mask):
    import ml_dtypes

    from concourse.bass_utils import run_bass_kernel_spmd

    B, H, L, D = q.shape
    U = B * H
    NCH = L // KC
    CPB = QB // KC
    NPAIR = (NCH + 1) // 2
    VO_W = 80
    if (
        U % N_CORES != 0
        or L % QB != 0
        or D != 64
        or not _causal_ok(att_mask, L)
    ):
        return _reference_np(q, k, v, att_mask, pad_mask)

    NH = U // N_CORES  # units (heads) per core

    pad = np.asarray(pad_mask, dtype=bool)          # [B, L]
    pad_u = np.repeat(pad, H, axis=0)               # [U, L]

    skip = frozenset(
        kc for kc in range(NCH)
        if np.all(pad_u[:, kc * KC:(kc + 1) * KC])
    )
    per_u_skip = [
        frozenset(
            kc for kc in range(NCH)
            if np.all(pad_u[u, kc * KC:(kc + 1) * KC])
        )
        for u in range(U)
    ]
    # chunks 0..CPB-1 must exist (qb0 bf16 path assumes them)
    if any(c in skip for c in range(CPB)) or any(
        s != skip for s in per_u_skip
    ):
        return _reference_np(q, k, v, att_mask, pad_mask)

    bf = ml_dtypes.bfloat16
    e4 = ml_dtypes.float8_e4m3

    qf = np.ascontiguousarray(
        q.reshape(U, L, D).transpose(0, 2, 1)
    ).astype(bf)
    kf = np.ascontiguousarray(
        k.reshape(U, L, D).transpose(0, 2, 1)
    ).astype(bf)

    # [V | 1] with padded keys zeroed
    vo = np.empty((U, L, D + 1), dtype=np.float32)
    vo[:, :, 0:D] = v.reshape(U, L, D)
    vo[:, :, D] = 1.0
    vo[pad_u] = 0.0
    voc = vo.reshape(U, NCH, KC, D + 1)             # [U, c, p, d]

    # fp8 paired weights: [U, p(=KC), pair, j, VO_W]
    vo8 = np.zeros((U, KC, NPAIR, 2, VO_W), dtype=np.float32)
    for pr in range(NPAIR):
        for j in (0, 1):
            c = 2 * pr + j
            if c < NCH and c not in skip:
                vo8[:, :, pr, j, 0:D + 1] = voc[:, c]
    vo8 = np.clip(vo8, -240.0, 240.0).astype(e4)

    # bf16 qb0 weights: [U, p, c(0..3), D+1]
    vo0 = np.ascontiguousarray(
        voc.transpose(0, 2, 1, 3)                   # [U, p, c, D+1]
    ).astype(bf)

    negi = (np.eye(KC, dtype=np.float32) * np.float32(-1e9)).astype(bf)
    u1 = (np.arange(KC)[None, :] < np.arange(KC)[:, None])
    utri = np.concatenate([u1, u1], axis=1).astype(bf)

    in_maps = []
    for c in range(N_CORES):
        sl = slice(c * NH, (c + 1) * NH)
        in_maps.append({
            "qt": qf[sl], "kt": kf[sl],
            "vo8": vo8[sl], "vo0": vo0[sl],
            "negi": negi, "utri": utri,
        })

    nc = _get_program(NH, L, D, skip)
    import os

    kwargs = {}
    if os.environ.get("BASS_KERNEL_PROFILE") == "1":
        kwargs = dict(trace=True, trace_cores=[0], stitch_traces=False)
    res = run_bass_kernel_spmd(nc, in_maps, list(range(N_CORES)), **kwargs)
    global LAST_RESULT
    LAST_RESULT = res
    raw = np.concatenate(
        [r["out"].astype(np.float32) for r in res.results], axis=0
    )
    # raw: [U, D+1, L] unnormalized -- normalize + transpose on host
    num = raw[:, 0:D, :]                            # [U, D, L]
    den = raw[:, D:D + 1, :]                        # [U, 1, L]
    out = (num / den).transpose(0, 2, 1)            # [U, L, D]
    out = np.ascontiguousarray(out).reshape(B, H, L, D)
    return out.astype(q.dtype, copy=False)


LAST_RESULT = None
